# revision 12
# baseline (speedup 1.0000x reference)
"""Trainium2 Bass kernel for nn_GedLayer (graph edit distance forward).

Math: the reference builds C, a N x N (N = 9216) cost matrix whose entries are
a 4x4 lookup T[A1[i,j], A2[k,l]] over edge-label pairs, then computes
    ged = 0.5 * v @ (Dmat @ v) + c @ v
with v = vec(S) from a 10-iteration Sinkhorn on the 96x96 node-cost grid.

Because the edge labels take only 4 values, the quadratic form factorizes:
    v' C0 v = sum_q  C' (sum_q G_q * B2_q) C   with  G_q = (P_q' S)' S
computed entirely with 96x96 matmuls -- no 9216^2 matrix is ever formed.
The Sinkhorn iteration is run in vector form (R = 1/(S0 C), C = 1/(S0' R)),
so each half-iteration is one 96x96x1 matvec on the PE plus a reciprocal.

Sharding across the 8 cores: the problem is one graph pair and the Sinkhorn
recursion is strictly serial, so the computation is latency-bound at 96x96
scale; it is replicated on all 8 cores (core 0's output is returned).
"""

import numpy as np
from contextlib import ExitStack

import concourse.bass as bass
import concourse.tile as tile
from concourse import mybir
from concourse.bass_utils import run_bass_kernel_spmd

NB_LABELS = 10
NB_EDGE_LABELS = 3
SINKHORN_ITERS = 10
L = NB_EDGE_LABELS + 1  # edge label alphabet {0 = no edge, 1..3}
N1 = 96                 # n + 1 = m + 1 for the fixed n = m = 95 problem size
F32 = mybir.dt.float32
N_CORES = 8

_NC_CACHE = {}


def _legalize_waits(nc):
    """Split multi-sem waits into standalone EventSemaphore instructions.

    This walrus build's codegen fits exactly one sync wait on each lowered
    instruction struct ("Too many sync wait commands" otherwise), but every
    engine queue accepts standalone single-wait EventSemaphore instructions
    (what raw-bass wait_ge emits). Hoist all but the last wait of any
    multi-wait instruction into such nops placed just before it.
    """
    n = 0
    for f in nc.m.functions:
        for bb in f.blocks:
            out = []
            for ins in bb.instructions:
                si = ins.sync_info
                waits = list(si.on_wait) if (si and si.on_wait) else []
                if len(waits) > 1:
                    for w in waits[:-1]:
                        n += 1
                        out.append(mybir.InstEventSemaphore(
                            name=f"LW-{n}",
                            engine=ins.engine,
                            ins=[],
                            outs=[],
                            sync_info=mybir.SyncInfo(on_wait=[w], on_update=[]),
                        ))
                    si.on_wait = [waits[-1]]
                out.append(ins)
            bb.instructions = out
    return n


def _build_nc(legalize=True):
    nc = bass.Bass()
    cg_d = nc.dram_tensor("cgrid", [N1, N1], F32, kind="ExternalInput")
    cgm_d = nc.dram_tensor("cgmod", [N1, N1], F32, kind="ExternalInput")
    cgTm_d = nc.dram_tensor("cgTmod", [N1, N1], F32, kind="ExternalInput")
    b2_d = nc.dram_tensor("b2", [N1, L, N1], F32, kind="ExternalInput")    # [k,q,l]
    pm_d = nc.dram_tensor("pmat", [N1, L, N1], F32, kind="ExternalInput")  # [j,q,i]
    dd_d = nc.dram_tensor("ddiag", [N1, N1], F32, kind="ExternalInput")
    out_d = nc.dram_tensor("out", [1, 1], F32, kind="ExternalOutput")

    ExpF = mybir.ActivationFunctionType.Exp
    mult = mybir.AluOpType.mult
    add = mybir.AluOpType.add

    with tile.TileContext(nc) as tc, ExitStack() as ctx:
        sb = ctx.enter_context(tc.tile_pool(name="sb", bufs=1))

        cg = sb.tile([N1, N1], F32)
        nc.sync.dma_start(out=cg[:], in_=cg_d[:])
        cgm = sb.tile([N1, N1], F32)
        nc.sync.dma_start(out=cgm[:], in_=cgm_d[:])
        cgTm = sb.tile([N1, N1], F32)
        nc.sync.dma_start(out=cgTm[:], in_=cgTm_d[:])
        b2 = sb.tile([N1, L, N1], F32)
        nc.sync.dma_start(out=b2[:], in_=b2_d[:])
        pm = sb.tile([N1, L, N1], F32)
        nc.sync.dma_start(out=pm[:], in_=pm_d[:])
        dd = sb.tile([N1, N1], F32)
        nc.sync.dma_start(out=dd[:], in_=dd_d[:])

        ones_col = sb.tile([N1, 1], F32)
        nc.vector.memset(ones_col[:], 1.0)

        # S0 = exp(-0.5*cgrid); s0m/s0Tm = exp of the column-95-pinned grids
        # used as Sinkhorn matvec operands (their column 95 is e_95, which
        # makes u[95] = C[95] = 1 so a full-tile reciprocal keeps the pin).
        s0a = sb.tile([N1, N1], F32)
        nc.scalar.activation(out=s0a[:], in_=cg[:], func=ExpF, scale=-0.5)
        s0ma = sb.tile([N1, N1], F32)
        nc.scalar.activation(out=s0ma[:], in_=cgm[:], func=ExpF, scale=-0.5)
        s0Tma = sb.tile([N1, N1], F32)
        nc.scalar.activation(out=s0Tma[:], in_=cgTm[:], func=ExpF, scale=-0.5)

        # Funnel every matmul operand through the vector engine so each PE
        # instruction waits on a single semaphore (walrus's PE LW struct
        # rejects multi-sem waits: "Too many sync wait commands").
        s0 = sb.tile([N1, N1], F32)
        nc.vector.tensor_copy(out=s0[:], in_=s0a[:])
        s0m = sb.tile([N1, N1], F32)
        nc.vector.tensor_copy(out=s0m[:], in_=s0ma[:])
        s0Tm = sb.tile([N1, N1], F32)
        nc.vector.tensor_copy(out=s0Tm[:], in_=s0Tma[:])
        pmd = sb.tile([N1, L, N1], F32)
        nc.vector.tensor_copy(out=pmd[:], in_=pm[:])
        b2d = sb.tile([N1, L, N1], F32)
        nc.vector.tensor_copy(out=b2d[:], in_=b2[:])

        # Sinkhorn in cumulative-scale vector form. The eps_assign2 "last
        # row/col scale pinned to 1" rule makes R[95] = C[95] = 1 invariant,
        # so each iteration writes elements 0..94 and pins element 95 to 1.
        # R/C get a FRESH tile every iteration: rewriting one tile would add a
        # same-engine WAW wait on top of the cross-engine RAW wait, and
        # walrus's lowered-instruction structs only fit one sync wait.
        rc = ctx.enter_context(tc.tile_pool(name="rc", bufs=3))
        Cv = rc.tile([N1, 1], F32, tag="c")
        nc.vector.memset(Cv[:], 1.0)
        Rv = None

        with tc.tile_pool(name="mv", bufs=2, space="PSUM") as mv:
            for _ in range(SINKHORN_ITERS):
                u = mv.tile([N1, 1], F32, tag="mv")
                nc.tensor.matmul(u[:], lhsT=s0Tm[:], rhs=Cv[:], start=True, stop=True)
                Rv = rc.tile([N1, 1], F32, tag="r")
                nc.vector.reciprocal(out=Rv[:], in_=u[:])
                w = mv.tile([N1, 1], F32, tag="mv")
                nc.tensor.matmul(w[:], lhsT=s0m[:], rhs=Rv[:], start=True, stop=True)
                Cv = rc.tile([N1, 1], F32, tag="c")
                nc.vector.reciprocal(out=Cv[:], in_=w[:])

        # S' = diag(R) S0; the C scaling is folded into the final reductions.
        sp = sb.tile([N1, N1], F32)
        nc.vector.tensor_scalar_mul(sp[:], s0[:], Rv[:])

        # G'' = sum_q (P_q' S')' S' .* B2_q   (all 96x96 matmuls)
        gacc = sb.tile([N1, N1], F32)
        with tc.tile_pool(name="qg", bufs=3, space="PSUM") as qg, \
                tc.tile_pool(name="zsb", bufs=2) as zsb, \
                tc.tile_pool(name="gt", bufs=2) as gt:
            for q in range(L):
                z_ps = qg.tile([N1, N1], F32, tag="qg")
                nc.tensor.matmul(z_ps[:], lhsT=pmd[:, q, :], rhs=sp[:],
                                 start=True, stop=True)
                z_sb = zsb.tile([N1, N1], F32, tag="z")
                nc.vector.tensor_copy(out=z_sb[:], in_=z_ps[:])
                g_ps = qg.tile([N1, N1], F32, tag="qg")
                nc.tensor.matmul(g_ps[:], lhsT=z_sb[:], rhs=sp[:],
                                 start=True, stop=True)
                if q == 0:
                    nc.vector.tensor_mul(gacc[:], g_ps[:], b2d[:, q, :])
                else:
                    g_sb = gt.tile([N1, N1], F32, tag="gt")
                    nc.vector.tensor_mul(g_sb[:], g_ps[:], b2d[:, q, :])
                    nc.vector.tensor_add(gacc[:], gacc[:], g_sb[:])

        # cterm matrix cgrid .* S'; dterm matrix ddiag .* S'^2
        cs = sb.tile([N1, N1], F32)
        nc.vector.tensor_mul(cs[:], cg[:], sp[:])
        ds = sb.tile([N1, N1], F32)
        nc.vector.tensor_mul(ds[:], sp[:], sp[:])
        nc.vector.tensor_mul(ds[:], ds[:], dd[:])

        with tc.tile_pool(name="red", bufs=2, space="PSUM") as red, \
                tc.tile_pool(name="cols", bufs=1) as cols:
            # gv[l] = sum_k G''[k,l] C[k];  qcol = gv .* C  (so sum qcol = C'G''C)
            gv_ps = red.tile([N1, 1], F32, tag="red")
            nc.tensor.matmul(gv_ps[:], lhsT=gacc[:], rhs=Cv[:], start=True, stop=True)
            qcol = cols.tile([N1, 1], F32)
            nc.vector.tensor_mul(qcol[:], gv_ps[:], Cv[:])

            # ccol[k] = (sum_j cs[j,k]) * C[k]
            cs_ps = red.tile([N1, 1], F32, tag="red")
            nc.tensor.matmul(cs_ps[:], lhsT=cs[:], rhs=ones_col[:], start=True, stop=True)
            ccol = cols.tile([N1, 1], F32)
            nc.vector.tensor_mul(ccol[:], cs_ps[:], Cv[:])

            # dcol[k] = (sum_j ds[j,k]) * C[k]^2
            ds_ps = red.tile([N1, 1], F32, tag="red")
            nc.tensor.matmul(ds_ps[:], lhsT=ds[:], rhs=ones_col[:], start=True, stop=True)
            dcol = cols.tile([N1, 1], F32)
            nc.vector.tensor_mul(dcol[:], ds_ps[:], Cv[:])
            nc.vector.tensor_mul(dcol[:], dcol[:], Cv[:])

            # comb = 0.5*qcol + ccol - 0.5*dcol;  ged = sum_k comb[k]
            comb = cols.tile([N1, 1], F32)
            nc.vector.scalar_tensor_tensor(out=comb[:], in0=qcol[:], scalar=0.5,
                                           in1=ccol[:], op0=mult, op1=add)
            nc.vector.scalar_tensor_tensor(out=comb[:], in0=dcol[:], scalar=-0.5,
                                           in1=comb[:], op0=mult, op1=add)

            tot_ps = red.tile([1, 1], F32, tag="tot")
            nc.tensor.matmul(tot_ps[:], lhsT=comb[:], rhs=ones_col[:],
                             start=True, stop=True)
            out_sb = cols.tile([1, 1], F32)
            nc.vector.tensor_copy(out=out_sb[:], in_=tot_ps[:])
            nc.sync.dma_start(out=out_d[:], in_=out_sb[:])

    if legalize:
        _legalize_waits(nc)  # hardware path only; CoreSim rejects bare nops
    return nc


def _host_prep(node_weights, edge_weights, A_g1, A_g2, labels1, labels2, n, m):
    n = int(n)
    m = int(m)
    n1, m1 = n + 1, m + 1
    assert n1 == N1 and m1 == N1, (n, m)

    cn = np.maximum(np.asarray(node_weights, np.float32), 0)
    ce = np.maximum(np.asarray(edge_weights, np.float32), 0)
    node_ins_del = cn[-1]
    edge_ins_del = ce[-1]
    node_costs = np.zeros((NB_LABELS, NB_LABELS), np.float32)
    node_costs[np.triu_indices(NB_LABELS, 1)] = cn[:-1]
    node_costs = node_costs + node_costs.T
    edge_costs = np.zeros((NB_EDGE_LABELS, NB_EDGE_LABELS), np.float32)
    edge_costs[np.triu_indices(NB_EDGE_LABELS, 1)] = ce[:-1]
    edge_costs = edge_costs + edge_costs.T

    A1 = np.zeros((n1, n1), np.int32)
    A1[:n, :n] = np.asarray(A_g1)[:n * n].reshape(n, n)
    A2 = np.zeros((m1, m1), np.int32)
    A2[:m, :m] = np.asarray(A_g2)[:m * m].reshape(m, m)

    # 4x4 edge-pair cost table: xor ins/del term + substitution term
    T = np.zeros((L, L), np.float32)
    for a1 in range(L):
        for a2 in range(L):
            v = np.float32(0.0)
            if (a1 != 0) != (a2 != 0):
                v += edge_ins_del
            if a1 >= 1 and a2 >= 1:
                v += edge_costs[a1 - 1, a2 - 1]
            T[a1, a2] = v

    b2 = np.empty((m1, L, m1), np.float32)          # [k,q,l] = 1[A2[k,l]==q]
    for q in range(L):
        b2[:, q, :] = (A2 == q)
    TA1 = T[A1]                                      # [i,j,q] = T[A1[i,j],q]
    pmat = np.ascontiguousarray(TA1.transpose(1, 2, 0))  # [j,q,i]

    Dnm = node_costs[np.asarray(labels1)[:n][:, None], np.asarray(labels2)[:m][None, :]]
    cgrid = np.full((n1, m1), node_ins_del, np.float32)
    cgrid[:n, :m] = Dnm
    cgrid[n, m] = 0.0

    ddiag = T[A1.diagonal()[:, None], A2.diagonal()[None, :]].astype(np.float32)

    # Sinkhorn matvec operands with column 95 pinned to e_95: exp(-0.5*BIG)
    # underflows to exactly 0, and cgrid[95,95] = 0 gives exp(0) = 1, so the
    # matvecs yield u[95] = C[95] and w[95] = R[95] (the "scale fixed to 1"
    # rule) without any partial-partition writes on device.
    BIG = np.float32(1e4)
    cgmod = cgrid.copy()
    cgmod[:, m1 - 1] = BIG
    cgmod[n1 - 1, m1 - 1] = 0.0
    cgTmod = np.ascontiguousarray(cgrid.T)
    cgTmod[:, n1 - 1] = BIG
    cgTmod[m1 - 1, n1 - 1] = 0.0

    return {
        "cgrid": np.ascontiguousarray(cgrid),
        "cgmod": np.ascontiguousarray(cgmod),
        "cgTmod": np.ascontiguousarray(cgTmod),
        "b2": np.ascontiguousarray(b2),
        "pmat": pmat,
        "ddiag": np.ascontiguousarray(ddiag),
    }


def run(inputs, trace=False, **spmd_kwargs):
    """Build (cached) and execute the SPMD kernel; returns (value, BassKernelResults)."""
    in_map = _host_prep(**inputs)
    if "nc" not in _NC_CACHE:
        _NC_CACHE["nc"] = _build_nc()
    nc = _NC_CACHE["nc"]
    core_ids = list(range(N_CORES))
    res = run_bass_kernel_spmd(
        nc, [dict(in_map) for _ in core_ids], core_ids, trace=trace, **spmd_kwargs
    )
    val = np.float32(res.results[0]["out"].reshape(()))
    return val, res


def kernel(**inputs) -> np.ndarray:
    val, _ = run(inputs)
    return np.asarray(val, np.float32).reshape(())


# revision 14
# speedup vs baseline: 1.0491x; 1.0491x over previous
"""Trainium2 Bass kernel for nn_GedLayer (graph edit distance forward).

The reference builds a 9216x9216 cost matrix C whose entries are a 4x4
lookup T[A1[i,j], A2[k,l]] over edge-label pairs, then computes
    ged = 0.5 * v @ (Dmat @ v) + c @ v
with v = vec(S) from a 10-iteration Sinkhorn on the 96x96 node-cost grid.

Because edge labels take only 4 values, the quadratic form factorizes into
96x96 matmuls (no 9216^2 matrix is ever formed):
    Zt[k,(q,i)] = sum_j S'[j,k] P_q[j,i]          one wide 96x96x384 matmul
    F[i,l]      = sum_qk Zt[k,(q,i)] C[k] B2_q[k,l]   4 PSUM-accum matmuls
    v' C0 v     = sum_il F[i,l] S'[i,l] C[l]
where P_q/B2_q are host-built indicator lookups of the int edge matrices,
S' = diag(R) S0, and (R, C) come from Sinkhorn run in vector form
(R = 1/(S0m' C), C = 1/(S0Tm' R); the "last scale pinned to 1" rule is
implemented by baking an e_95 column into the matvec operands so a
full-tile reciprocal preserves the pin). All arithmetic on device is fp32.

Sharding: one graph pair, strictly serial Sinkhorn recursion -> the
problem is latency-bound at 96x96 scale, so the computation is replicated
on all 8 cores (SPMD) and core 0's output is returned.
"""

import numpy as np
from contextlib import ExitStack

import concourse.bass as bass
import concourse.tile as tile
from concourse import mybir
from concourse.bass_utils import run_bass_kernel_spmd

NB_LABELS = 10
NB_EDGE_LABELS = 3
SINKHORN_ITERS = 10
L = NB_EDGE_LABELS + 1
N1 = 96
F32 = mybir.dt.float32
N_CORES = 8

_NC_CACHE = {}


def _legalize_waits(nc):
    """Split multi-sem waits into standalone EventSemaphore instructions
    (this walrus codegen fits one sync wait per lowered instruction)."""
    n = 0
    for f in nc.m.functions:
        for bb in f.blocks:
            out = []
            for ins in bb.instructions:
                si = ins.sync_info
                waits = list(si.on_wait) if (si and si.on_wait) else []
                if len(waits) > 1:
                    for w in waits[:-1]:
                        n += 1
                        out.append(mybir.InstEventSemaphore(
                            name=f"LW-{n}",
                            engine=ins.engine,
                            ins=[],
                            outs=[],
                            sync_info=mybir.SyncInfo(on_wait=[w], on_update=[]),
                        ))
                    si.on_wait = [waits[-1]]
                out.append(ins)
            bb.instructions = out
    return n


def _build_nc(legalize=True):
    nc = bass.Bass()
    # grids = [cgrid | cgmod | cgTmod | ddiag] along the free dim
    g_d = nc.dram_tensor("grids", [N1, 4, N1], F32, kind="ExternalInput")
    # tabs = [pmat (j,q,i) | b2 (k,q,l)] along the free dim
    t_d = nc.dram_tensor("tabs", [N1, 2, L, N1], F32, kind="ExternalInput")
    out_d = nc.dram_tensor("out", [1, 1], F32, kind="ExternalOutput")

    ExpF = mybir.ActivationFunctionType.Exp
    mult = mybir.AluOpType.mult
    add = mybir.AluOpType.add

    with tile.TileContext(nc) as tc, ExitStack() as ctx:
        sb = ctx.enter_context(tc.tile_pool(name="sb", bufs=1))

        grids = sb.tile([N1, 4, N1], F32)
        nc.sync.dma_start(out=grids[:], in_=g_d[:])
        tabs = sb.tile([N1, 2, L, N1], F32)
        nc.scalar.dma_start(out=tabs[:], in_=t_d[:])
        cg = grids[:, 0, :]
        cgm = grids[:, 1, :]
        cgTm = grids[:, 2, :]
        dd = grids[:, 3, :]
        pmall = tabs[:, 0, :, :].rearrange("p q i -> p (q i)")
        b2all = tabs[:, 1, :, :]

        ones_col = sb.tile([N1, 1], F32)
        nc.vector.memset(ones_col[:], 1.0)

        s0 = sb.tile([N1, N1], F32)
        nc.scalar.activation(out=s0[:], in_=cg, func=ExpF, scale=-0.5)
        s0m = sb.tile([N1, N1], F32)
        nc.scalar.activation(out=s0m[:], in_=cgm, func=ExpF, scale=-0.5)
        s0Tm = sb.tile([N1, N1], F32)
        nc.scalar.activation(out=s0Tm[:], in_=cgTm, func=ExpF, scale=-0.5)

        # Sinkhorn (see kernel.py): fresh R/C tiles per iteration, pin via
        # the e_95 column baked into cgmod/cgTmod.
        rc = ctx.enter_context(tc.tile_pool(name="rc", bufs=3))
        Cv = rc.tile([N1, 1], F32, tag="c")
        nc.vector.memset(Cv[:], 1.0)
        Rv = None

        with tc.tile_pool(name="mv", bufs=2, space="PSUM") as mv:
            for _ in range(SINKHORN_ITERS):
                u = mv.tile([N1, 1], F32, tag="mv")
                nc.tensor.matmul(u[:], lhsT=s0Tm[:], rhs=Cv[:], start=True, stop=True)
                Rv = rc.tile([N1, 1], F32, tag="r")
                nc.vector.reciprocal(out=Rv[:], in_=u[:])
                w = mv.tile([N1, 1], F32, tag="mv")
                nc.tensor.matmul(w[:], lhsT=s0m[:], rhs=Rv[:], start=True, stop=True)
                Cv = rc.tile([N1, 1], F32, tag="c")
                nc.vector.reciprocal(out=Cv[:], in_=w[:])

        # S' = diag(R) S0; b2c = B2 scaled by C on the k (partition) axis
        sp = sb.tile([N1, N1], F32)
        nc.vector.tensor_scalar_mul(sp[:], s0[:], Rv[:])
        b2c = sb.tile([N1, L, N1], F32)
        nc.vector.tensor_scalar_mul(b2c[:], b2all, Cv[:])

        # Zt[k,(q,i)] = sum_j S'[j,k] P_q[j,i]   (one wide matmul)
        # F[i,l]     = sum_q sum_k Zt[k,(q,i)] C[k] B2_q[k,l]  (PSUM-accum)
        # Q          = sum_il F[i,l] S'[i,l] C[l]
        with tc.tile_pool(name="zt", bufs=1, space="PSUM") as ztp, \
                tc.tile_pool(name="fp", bufs=1, space="PSUM") as fpp, \
                tc.tile_pool(name="zsb", bufs=1) as zsb:
            zt_ps = ztp.tile([N1, L, N1], F32)
            nc.tensor.matmul(zt_ps[:].rearrange("p q i -> p (q i)"),
                             lhsT=sp[:], rhs=pmall, start=True, stop=True)
            zt = zsb.tile([N1, L, N1], F32)
            nc.vector.tensor_copy(out=zt[:], in_=zt_ps[:])

            f_ps = fpp.tile([N1, N1], F32)
            for q in range(L):
                nc.tensor.matmul(f_ps[:], lhsT=zt[:, q, :], rhs=b2c[:, q, :],
                                 start=(q == 0), stop=(q == L - 1))

            fs = sb.tile([N1, N1], F32)
            nc.vector.tensor_mul(fs[:], f_ps[:], sp[:])

        cs = sb.tile([N1, N1], F32)
        nc.vector.tensor_mul(cs[:], cg, sp[:])
        ds = sb.tile([N1, N1], F32)
        nc.vector.tensor_mul(ds[:], sp[:], sp[:])
        nc.vector.tensor_mul(ds[:], ds[:], dd)

        with tc.tile_pool(name="red", bufs=2, space="PSUM") as red, \
                tc.tile_pool(name="cols", bufs=1) as cols:
            q_ps = red.tile([N1, 1], F32, tag="red")
            nc.tensor.matmul(q_ps[:], lhsT=fs[:], rhs=ones_col[:], start=True, stop=True)
            qcol = cols.tile([N1, 1], F32)
            nc.vector.tensor_mul(qcol[:], q_ps[:], Cv[:])

            c_ps = red.tile([N1, 1], F32, tag="red")
            nc.tensor.matmul(c_ps[:], lhsT=cs[:], rhs=ones_col[:], start=True, stop=True)
            ccol = cols.tile([N1, 1], F32)
            nc.vector.tensor_mul(ccol[:], c_ps[:], Cv[:])

            d_ps = red.tile([N1, 1], F32, tag="red")
            nc.tensor.matmul(d_ps[:], lhsT=ds[:], rhs=ones_col[:], start=True, stop=True)
            dcol = cols.tile([N1, 1], F32)
            nc.vector.tensor_mul(dcol[:], d_ps[:], Cv[:])
            nc.vector.tensor_mul(dcol[:], dcol[:], Cv[:])

            comb = cols.tile([N1, 1], F32)
            nc.vector.scalar_tensor_tensor(out=comb[:], in0=qcol[:], scalar=0.5,
                                           in1=ccol[:], op0=mult, op1=add)
            nc.vector.scalar_tensor_tensor(out=comb[:], in0=dcol[:], scalar=-0.5,
                                           in1=comb[:], op0=mult, op1=add)

            tot_ps = red.tile([1, 1], F32, tag="tot")
            nc.tensor.matmul(tot_ps[:], lhsT=comb[:], rhs=ones_col[:],
                             start=True, stop=True)
            out_sb = cols.tile([1, 1], F32)
            nc.vector.tensor_copy(out=out_sb[:], in_=tot_ps[:])
            nc.sync.dma_start(out=out_d[:], in_=out_sb[:])

    if legalize:
        _legalize_waits(nc)
    return nc


def _host_prep(node_weights, edge_weights, A_g1, A_g2, labels1, labels2, n, m):
    n = int(n)
    m = int(m)
    n1, m1 = n + 1, m + 1
    assert n1 == N1 and m1 == N1, (n, m)

    cn = np.maximum(np.asarray(node_weights, np.float32), 0)
    ce = np.maximum(np.asarray(edge_weights, np.float32), 0)
    node_ins_del = cn[-1]
    edge_ins_del = ce[-1]
    node_costs = np.zeros((NB_LABELS, NB_LABELS), np.float32)
    node_costs[np.triu_indices(NB_LABELS, 1)] = cn[:-1]
    node_costs = node_costs + node_costs.T
    edge_costs = np.zeros((NB_EDGE_LABELS, NB_EDGE_LABELS), np.float32)
    edge_costs[np.triu_indices(NB_EDGE_LABELS, 1)] = ce[:-1]
    edge_costs = edge_costs + edge_costs.T

    A1 = np.zeros((n1, n1), np.int32)
    A1[:n, :n] = np.asarray(A_g1)[:n * n].reshape(n, n)
    A2 = np.zeros((m1, m1), np.int32)
    A2[:m, :m] = np.asarray(A_g2)[:m * m].reshape(m, m)

    T = np.zeros((L, L), np.float32)
    for a1 in range(L):
        for a2 in range(L):
            v = np.float32(0.0)
            if (a1 != 0) != (a2 != 0):
                v += edge_ins_del
            if a1 >= 1 and a2 >= 1:
                v += edge_costs[a1 - 1, a2 - 1]
            T[a1, a2] = v

    b2 = np.empty((m1, L, m1), np.float32)           # [k,q,l]
    for q in range(L):
        b2[:, q, :] = (A2 == q)
    TA1 = T[A1]                                       # [i,j,q]
    pmat = np.ascontiguousarray(TA1.transpose(1, 2, 0))  # [j,q,i]

    Dnm = node_costs[np.asarray(labels1)[:n][:, None], np.asarray(labels2)[:m][None, :]]
    cgrid = np.full((n1, m1), node_ins_del, np.float32)
    cgrid[:n, :m] = Dnm
    cgrid[n, m] = 0.0

    ddiag = T[A1.diagonal()[:, None], A2.diagonal()[None, :]].astype(np.float32)

    BIG = np.float32(1e4)
    cgmod = cgrid.copy()
    cgmod[:, m1 - 1] = BIG
    cgmod[n1 - 1, m1 - 1] = 0.0
    cgTmod = np.ascontiguousarray(cgrid.T)
    cgTmod[:, n1 - 1] = BIG
    cgTmod[m1 - 1, n1 - 1] = 0.0

    grids = np.stack([cgrid, cgmod, cgTmod, ddiag], axis=1)  # [96, 4, 96]
    tabs = np.stack([pmat, b2], axis=1)                      # [96, 2, L, 96]

    return {
        "grids": np.ascontiguousarray(grids),
        "tabs": np.ascontiguousarray(tabs),
    }


def run(inputs, trace=False, **spmd_kwargs):
    in_map = _host_prep(**inputs)
    if "nc" not in _NC_CACHE:
        _NC_CACHE["nc"] = _build_nc()
    nc = _NC_CACHE["nc"]
    core_ids = list(range(N_CORES))
    res = run_bass_kernel_spmd(
        nc, [dict(in_map) for _ in core_ids], core_ids, trace=trace, **spmd_kwargs
    )
    val = np.float32(res.results[0]["out"].reshape(()))
    return val, res


def kernel(**inputs) -> np.ndarray:
    val, _ = run(inputs)
    return np.asarray(val, np.float32).reshape(())
